# revision 34
# baseline (speedup 1.0000x reference)
"""Trainium2 Bass kernel for Luong local-p sparse attention.

Math (per batch n, full shapes N=64, L=258, H=1024, Q=256):
    score = (h_t @ W_a) @ enc^T           masked to window [p_t-16, p_t+16]
    align = softmax(score) * gauss(p_t)
    out   = tanh([align @ enc, h_t] @ W_c^T)

Only a 33-wide window of enc survives the mask, so the kernel gathers
windows host-side and pushes W_a / W_c[:, :H] through the 33-wide side:
    u  = W_a-transform of window   (uT[h', (n,j)] = sum_h W_aT[h,h'] enc_w[(n,j),h])
    s  = uT^T-partial scores       (score^T[j, q] = sum_h' uT[h',j] h_t[q,h'])
    softmax over j (33 rows) j-major with a 4th-power renormalization
    v  = W_c1-transform of window  (v[(n,j), h'] = sum_h enc_w[(n,j),h] W_c1T[h,h'])
    outT[h', (n,q)] = tanh(sum_h W_c2T[h,h'] dec[h,(n,q)] + sum_j v[j,h'] t[j,q])

The output GEMM runs TRANSPOSED (outT layout [H, B*Q]): stationary W_c2
chunks are shared across batches, the per-batch ctx matmuls (contraction
over the 33 window rows) accumulate into the same PSUM tiles, and tanh +
stores pipeline per 128-row output chunk (no big tail).

DMA priority order: enc -> W_a -> W_c1 -> dec b0,b1 -> W_c2 -> dec b2..b7,
with chunked loads so the PE starts ~4us in (warm-up matmuls burn off the
HAM cold clock during the DMA-bound preamble).

Precision: enc / dec / W_a / softmax stay fp32r (the exp() amplifies
absolute score error, so the score path needs the 11-bit mantissa);
W_c1 / W_c2 are bf16 *stationary/moving weight* operands only, and the
output is written bf16 (host upcasts) - each adds ~1e-3 rel err against
a 2e-2 budget while cutting HBM traffic 30.4 -> 22.1 MB.

Data parallel over batch: 8 batches per core x 8 cores.
"""

import numpy as np
import ml_dtypes

import concourse.bass as bass
import concourse.bacc as bacc
import concourse.mybir as mybir
import concourse.tile as tile
from concourse.bass_utils import run_bass_kernel_spmd

# Problem constants (hardcoded per harness contract).
N, L, H, Q = 64, 258, 1024, 256
WINDOW = 16.0
DEV_POW = 128.0
NCORES = 8
B = N // NCORES  # batches per core
W = 33           # window width (positions that can survive the mask)
HC = H // 128    # h-chunks of 128 (PE contraction tiles)
F32 = mybir.dt.float32
F32R = mybir.dt.float32r
BF16 = mybir.dt.bfloat16
AF = mybir.ActivationFunctionType

# exp is computed as t = exp(s/4 + bias); bias = LOG_ALPHA keeps the
# column-sum T = sum_j t below fp32 max.  alpha cancels in w = t/T.
LOG_ALPHA = -4.8520302  # -7*ln(2)
MASK_BIAS = -10000.0    # exp(<= -9900) == 0 in fp32

OUT_NAME = "outT"


def build_nc() -> bass.Bass:
    nc = bacc.Bacc()
    enc_wT = nc.declare_dram_parameter("enc_wT", [H, B * W], F32R, isOutput=False)
    dec_hT = nc.declare_dram_parameter("dec_hT", [H, B * Q], F32R, isOutput=False)
    W_aT = nc.declare_dram_parameter("W_aT", [H, H], F32R, isOutput=False)
    W_c1T = nc.declare_dram_parameter("W_c1T", [H, H], BF16, isOutput=False)
    Wc2P = nc.declare_dram_parameter("Wc2P", [128, HC * HC * 128], F32R, isOutput=False)
    biasT = nc.declare_dram_parameter("biasT", [W, B], F32, isOutput=False)
    gPackT = nc.declare_dram_parameter("gPackT", [3 * W, 3], F32, isOutput=False)
    onesD = nc.declare_dram_parameter("onesD", [W, W], F32R, isOutput=False)
    outT = nc.declare_dram_parameter(OUT_NAME, [H, B * Q], BF16, isOutput=True)

    enc_r = enc_wT[:, :].rearrange("(c p) m -> p c m", p=128)
    WaT_r = W_aT[:, :].rearrange("(c p) m -> p c m", p=128)
    Wc1_r = W_c1T[:, :].rearrange("(c p) m -> p c m", p=128)
    Wc2_r = Wc2P[:, :].rearrange("p (o c m) -> p o c m", o=HC, c=HC)
    dec_r = dec_hT[:, :].rearrange("(c p) (n q) -> p c n q", p=128, q=Q)
    outT_r = outT[:, :].rearrange("(o p) m -> p o m", p=128)

    with tile.TileContext(nc) as tc:
        with (
            tc.tile_pool(name="const", bufs=1) as cpool,
            tc.tile_pool(name="sm", bufs=1) as sm_pool,
            tc.tile_pool(name="vstp", bufs=2) as vstp,
            tc.tile_pool(name="outp", bufs=3) as outp,
            tc.tile_pool(name="psA", bufs=2, space="PSUM") as psA,
            tc.tile_pool(name="psG", bufs=6, space="PSUM") as psG,
        ):
            # ---------------- resident tensors ----------------
            enc_sb = cpool.tile([128, HC, B * W], F32R)
            enc_bf = cpool.tile([128, HC, B * W], BF16)
            WaT_sb = cpool.tile([128, HC, H], F32R)
            Wc1_sb = cpool.tile([128, HC, H], BF16)
            Wc2_sb = cpool.tile([128, HC, HC, 128], F32R)
            dec_sb = cpool.tile([128, HC, B, Q], F32R)
            uT_sb = cpool.tile([128, HC, B * W], F32R)
            v_sb = cpool.tile([W, B, H], BF16)
            bias_sb = cpool.tile([W, B], F32)
            gpack_sb = cpool.tile([3 * W, 3], F32)
            ones_sb = cpool.tile([W, W], F32R)
            wconst = cpool.tile([128, B * W], F32)

            # ---------------- DMA schedule ----------------
            # sync ring carries every input load in priority order; the
            # issue order IS the schedule.  scalar ring takes the tiny
            # consts so sync starts on enc immediately.
            nc.scalar.dma_start(out=bias_sb, in_=biasT[:, :])
            nc.scalar.dma_start(out=gpack_sb, in_=gPackT[:, :])
            nc.scalar.dma_start(out=ones_sb, in_=onesD[:, :])

            # enc and Wc1 interleaved: the v phase consumes (enc ck, Wc1
            # c[k/2]) pairs, so it starts as soon as the first pair lands.
            for kc in range(HC):
                nc.sync.dma_start(out=enc_sb[:, kc, :], in_=enc_r[:, kc, :])
                # bf16 shadow of enc for the all-bf16 v-phase matmuls
                nc.vector.tensor_copy(out=enc_bf[:, kc, :], in_=enc_sb[:, kc, :])
                if kc % 2 == 1:
                    i = kc // 2
                    nc.sync.dma_start(
                        out=Wc1_sb[:, 2 * i:2 * i + 2, :],
                        in_=Wc1_r[:, 2 * i:2 * i + 2, :],
                    )
            for kc in range(HC):
                nc.sync.dma_start(out=WaT_sb[:, kc, :], in_=WaT_r[:, kc, :])
            for n in range(2):
                nc.sync.dma_start(out=dec_sb[:, :, n, :], in_=dec_r[:, :, n, :])
            for o in range(HC):
                nc.sync.dma_start(out=Wc2_sb[:, o, :, :], in_=Wc2_r[:, o, :, :])
            for n in range(2, B):
                nc.sync.dma_start(out=dec_sb[:, :, n, :], in_=dec_r[:, :, n, :])

            # ---------------- PE warm-up ----------------
            # dummy matmuls on a memset const (no DMA dependency) trip the
            # HAM un-throttle during the DMA-bound preamble (never read).
            nc.gpsimd.memset(wconst, 1.0)
            wm = psG.tile([128, B * W], F32, tag="G", name="warm")
            for i in range(5):
                nc.tensor.matmul(
                    wm, lhsT=wconst[:, 0:128], rhs=wconst[:, :],
                    start=True, stop=True,
                )

            # ---------------- v phase (Wc1-paced) ----------------
            GROUPS = [(0, 99), (99, 99), (198, 66)]

            def v_group(gi):
                g0, glen = GROUPS[gi]
                nb = glen // W
                for nt in range(2):
                    pv = psG.tile([128, 512], F32, tag="G", name=f"pv{nt}_{gi}")
                    for kc in range(HC):
                        nc.tensor.matmul(
                            pv[:glen, :],
                            lhsT=enc_bf[:, kc, g0:g0 + glen],
                            rhs=Wc1_sb[:, kc, nt * 512:(nt + 1) * 512],
                            start=(kc == 0),
                            stop=(kc == HC - 1),
                        )
                    vst = vstp.tile([128, 512], BF16, tag="vst", name=f"vst{nt}_{gi}")
                    # evacuate + fold the gaussian in one op
                    nc.vector.tensor_scalar_mul(
                        vst[:glen, :], pv[:glen, :], gpack_sb[:glen, gi:gi + 1]
                    )
                    for off in range(nb):
                        n = gi * 3 + off
                        nc.scalar.dma_start(
                            out=v_sb[:, n, nt * 512:(nt + 1) * 512],
                            in_=vst[off * W:(off + 1) * W, :],
                        )

            # ---------------- u phase (kc-outer, chunk-paced) ----------------
            def u_phase():
                for half in range(2):
                    hcs = range(4 * half, 4 * half + 4)
                    pus = {
                        hc: psG.tile([128, B * W], F32, tag="G", name=f"pu{hc}")
                        for hc in hcs
                    }
                    for kc in range(HC):
                        for hc in hcs:
                            nc.tensor.matmul(
                                pus[hc],
                                lhsT=WaT_sb[:, kc, hc * 128:(hc + 1) * 128],
                                rhs=enc_sb[:, kc, :],
                                start=(kc == 0),
                                stop=(kc == HC - 1),
                            )
                    for hc in hcs:
                        # vector, not scalar: the scalar queue carries the
                        # v scatters and would delay the score path
                        nc.vector.tensor_copy(out=uT_sb[:, hc, :], in_=pus[hc])

            # ---------------- softmax chains (split into 3 PE steps) ------
            t_tiles = {}
            sc_state = {}

            def sc1(n):
                ps = psA.tile([W, Q], F32, tag="A", name=f"ps{n}")
                for hc in range(HC):
                    nc.tensor.matmul(
                        ps,
                        lhsT=uT_sb[:, hc, n * W:(n + 1) * W],
                        rhs=dec_sb[:, hc, n, :],
                        start=(hc == 0),
                        stop=(hc == HC - 1),
                    )
                t = sm_pool.tile([W, Q], F32R, tag="t", bufs=3, name=f"t{n}")
                nc.scalar.activation(
                    out=t, in_=ps, func=AF.Exp, bias=bias_sb[:, n:n + 1], scale=0.25
                )
                sc_state[n] = t

            def sc2(n):
                t = sc_state[n]
                pT = psA.tile([W, Q], F32, tag="A", name=f"pT{n}")
                nc.tensor.matmul(pT, lhsT=ones_sb[:], rhs=t, start=True, stop=True)
                rT = sm_pool.tile([W, Q], F32, tag="r", bufs=2, name=f"rT{n}")
                nc.vector.reciprocal_approx_fast(out=rT, in_=pT)
                nc.vector.tensor_mul(t, t, rT)
                nc.vector.tensor_mul(t, t, t)
                nc.vector.tensor_mul(t, t, t)

            def sc3(n):
                t = sc_state.pop(n)
                pZ = psA.tile([W, Q], F32, tag="A", name=f"pZ{n}")
                nc.tensor.matmul(pZ, lhsT=ones_sb[:], rhs=t, start=True, stop=True)
                rZ = sm_pool.tile([W, Q], F32, tag="r", bufs=2, name=f"rZ{n}")
                nc.vector.reciprocal_approx_fast(out=rZ, in_=pZ)
                # final normalize writes the bf16 copy the ctx matmuls consume
                t_bf = sm_pool.tile([W, Q], BF16, tag="tb", bufs=8, name=f"tb{n}")
                nc.vector.tensor_mul(t_bf, t, rZ)
                t_tiles[n] = t_bf

            # ---------------- output GEMM (transposed, pipelined) ----------
            queue = []

            def unit(p, o):
                # one full-width matmul per k: a second start=True to the
                # same PSUM bank resets the whole bank, so the k=0 write
                # must cover all 512 columns at once.
                po = psG.tile([128, 512], F32, tag="G", name=f"po{p}_{o}")
                for k in range(HC):
                    nc.tensor.matmul(
                        po,
                        lhsT=Wc2_sb[:, o, k, :],
                        rhs=dec_sb[:, k, 2 * p:2 * p + 2, :],
                        start=(k == 0),
                        stop=False,
                    )
                queue.append((p, o, po))

            def flush_one():
                p, o, po = queue.pop(0)
                for i in range(2):
                    b = 2 * p + i
                    nc.tensor.matmul(
                        po[:, i * 256:(i + 1) * 256],
                        lhsT=v_sb[:, b, o * 128:(o + 1) * 128],
                        rhs=t_tiles[b],
                        start=False,
                        stop=True,
                    )
                oT = outp.tile([128, 512], BF16, tag="o", name=f"oT{p}_{o}")
                nc.scalar.activation(out=oT, in_=po, func=AF.Tanh)
                if p >= 2:
                    eng = nc.sync
                else:
                    eng = nc.gpsimd if o % 2 == 0 else nc.scalar
                eng.dma_start(out=outT_r[:, o, p * 512:(p + 1) * 512], in_=oT)

            # front block: v (Wc1-paced) -> u (Wa-paced) -> first score
            # chains (b0/b1) -> GEMM.  Later score chains + dec casts are
            # emitted at fixed unit boundaries inside the GEMM.
            v_group(0)
            v_group(1)
            v_group(2)
            u_phase()
            sc1(0)
            sc2(0)
            sc1(1)
            sc3(0)
            sc2(1)
            sc3(1)

            actions = {}

            def at(p, o, fn):
                actions.setdefault((p, o), []).append(fn)

            # pair p's chains trail the dec DMA arrivals: batch 2p lands
            # only after the full Wc2 stream, so its chain starts at the
            # end of pair p-1
            for p in range(1, 4):
                a, b = 2 * p, 2 * p + 1
                at(p - 1, 7, lambda a=a: sc1(a))
                at(p, 0, lambda b=b: sc1(b))
                at(p, 1, lambda a=a: sc2(a))
                at(p, 2, lambda b=b: sc2(b))
                at(p, 3, lambda a=a: sc3(a))
                at(p, 4, lambda b=b: sc3(b))

            for p in range(4):
                for o in range(HC):
                    unit(p, o)
                    for act in actions.get((p, o), ()):
                        act()
                    if len(queue) > 5:
                        flush_one()
                    # drain the pipeline early in the last pair: the tail
                    # is then just the final unit's ctx+tanh+store chain
                    if p == 3 and o >= 3 and queue:
                        flush_one()
            while queue:
                flush_one()
    nc.compile()
    return nc


def round_f32r(a: np.ndarray) -> np.ndarray:
    """Round fp32 to fp32r (TF32-like: 11-bit mantissa, low 12 bits zero),
    round-to-nearest-even.  This is what the PE consumes in fp32r mode."""
    u = np.ascontiguousarray(a, dtype=np.float32).view(np.uint32)
    lsb = (u >> np.uint32(12)) & np.uint32(1)
    u = (u + np.uint32(0x7FF) + lsb) & np.uint32(0xFFFFF000)
    return u.view(np.float32)


def prepare_in_maps(inputs: dict) -> list[dict]:
    enc = np.asarray(inputs["encoder_outputs"], dtype=np.float32)
    dec = np.asarray(inputs["decoder_h_t"], dtype=np.float32)
    src_len = np.asarray(inputs["src_len"], dtype=np.int32)
    p_t = np.asarray(inputs["p_t"], dtype=np.float32)
    W_a = np.asarray(inputs["W_a"], dtype=np.float32)
    W_c = np.asarray(inputs["W_c"], dtype=np.float32)

    # Window bounds, computed with the same fp32 ops as the reference.
    attn_start = np.maximum(p_t - np.float32(WINDOW), np.float32(0.0))
    attn_end = np.minimum(p_t + np.float32(WINDOW), src_len.astype(np.float32))
    s = np.ceil(attn_start).astype(np.int64)
    s = np.minimum(s, L - W)  # keep the 33-slice in bounds
    idx = s[:, None] + np.arange(W)[None, :]
    idxf = idx.astype(np.float32)
    mask = (idxf < attn_start[:, None]) | (idxf > attn_end[:, None])
    bias = np.where(mask, np.float32(MASK_BIAS), np.float32(LOG_ALPHA)).astype(np.float32)
    g = np.exp(-((idxf - p_t[:, None]) ** 2) / np.float32(DEV_POW)).astype(np.float32)

    enc_w = round_f32r(enc[np.arange(N)[:, None], idx, :])  # [N, W, H]
    dec = round_f32r(dec)
    W_aT = round_f32r(W_a.T)
    W_c1T = W_c[:, :H].T.astype(ml_dtypes.bfloat16)
    # W_c2T packed o-major: [HC(o), 128(o_in)] blocks contiguous per
    # partition so each o-chunk is one dense DMA.
    W_c2T = round_f32r(W_c[:, H:].T)                         # [H(h), H(h')]
    Wc2P = np.ascontiguousarray(
        W_c2T.reshape(HC, 128, HC, 128).transpose(1, 2, 0, 3).reshape(128, HC * HC * 128)
    )

    in_maps = []
    for c in range(NCORES):
        bs = slice(c * B, (c + 1) * B)
        gc = g[bs]  # [B, W]
        gpack = np.zeros((3 * W, 3), dtype=np.float32)
        for n in range(B):
            gi, off = divmod(n, 3)
            gpack[off * W:(off + 1) * W, gi] = gc[n]
        in_maps.append({
            "enc_wT": np.ascontiguousarray(enc_w[bs].transpose(2, 0, 1).reshape(H, B * W)),
            "dec_hT": np.ascontiguousarray(dec[bs].transpose(2, 0, 1).reshape(H, B * Q)),
            "W_aT": W_aT,
            "W_c1T": W_c1T,
            "Wc2P": Wc2P,
            "biasT": np.ascontiguousarray(bias[bs].T),
            "onesD": np.ones((W, W), dtype=np.float32),
            "gPackT": gpack,
        })
    return in_maps


def assemble(results) -> np.ndarray:
    """[H, B*Q] bf16 per core -> full [N, Q, H] f32."""
    outs = [
        np.asarray(results[c][OUT_NAME]).astype(np.float32).T.reshape(B, Q, H)
        for c in range(NCORES)
    ]
    return np.concatenate(outs, axis=0)


_NC = None


def get_nc() -> bass.Bass:
    global _NC
    if _NC is None:
        _NC = build_nc()
    return _NC


def kernel(**inputs) -> np.ndarray:
    nc = get_nc()
    in_maps = prepare_in_maps(inputs)
    res = run_bass_kernel_spmd(nc, in_maps, list(range(NCORES)))
    return assemble(res.results)
